# revision 23
# baseline (speedup 1.0000x reference)
# Bass/Trainium2 kernel for MHConvAttention (B=16, C=128, H=W=64, NH=8, OUT=512)
# Data-parallel over batch: 8 cores x 2 samples each.
#
# Per-sample layout: channels (128) on SBUF partitions, flattened spatial (4096)
# on the free dim. Depthwise convs run as fp8 DoubleRow matmuls: two taps packed
# per PE cell (pair stride in the padded-row buffer is 80 B, a multiple of 16),
# tap-major over 4-chunk groups so LDWEIGHTS hides behind the 4 streams. The
# content-lambda path uses a transposed QKV GEMM; the ECA channel-attention is
# folded into the out-projection weights; output is DMA'd in bf16.
import os
import numpy as np
import ml_dtypes

B, C, H, W = 16, 128, 64, 64
NH, HD, WIN, OUT = 8, 16, 5, 512
N = H * W
NCORES = 8
SPC = B // NCORES          # samples per core
NC8 = N // 512             # 512-wide chunks per sample
NJ = N // 128              # 128-wide chunks (transposed GEMM)
SCALING = HD ** (-0.5)
PW = 80                    # padded row width (fp8 conv buffers); 80 % 16 == 0
PH = 72                    # padded rows
QS = 16.0                  # q is stored as q/QS; conv5 weights carry x QS
W3S = 8.0                  # cpe weights stored x W3S in fp8; dequant 1/W3S
# tap pairs (dy_a, dy_b, dx); dy_a == dy_b means a half-weight self-pair
PAIRS3 = [(a, b, dx) for dx in range(3) for (a, b) in ((0, 1), (2, 2))]
PAIRS5 = [(a, b, dx) for dx in range(5) for (a, b) in ((0, 1), (2, 3), (4, 4))]

_CACHE = {}


def _build_nc():
    import bass_rust
    import concourse.bass as bass
    import concourse.tile as tile
    import concourse.mybir as mybir
    from concourse import bacc

    f32 = mybir.dt.float32
    f32r = mybir.dt.float32r
    bf16 = mybir.dt.bfloat16
    f8 = mybir.dt.float8e4
    Alu = mybir.AluOpType
    Act = mybir.ActivationFunctionType
    DR = mybir.MatmulPerfMode.DoubleRow

    def r(ap):
        return ap.bitcast(f32r)

    nc = bacc.Bacc(trn_type="TRN2", target_bir_lowering=False, debug=False)

    src_d = nc.dram_tensor("src", [SPC, C, H, W], f32, kind="ExternalInput").ap()
    d3_d = nc.dram_tensor("d3", [C, len(PAIRS3), 2, C], f8, kind="ExternalInput").ap()
    d5_d = nc.dram_tensor("d5", [C, len(PAIRS5), 2, C], f8, kind="ExternalInput").ap()
    wq_d = nc.dram_tensor("wq", [C, C], bf16, kind="ExternalInput").ap()
    wv_d = nc.dram_tensor("wv", [C, C], bf16, kind="ExternalInput").ap()
    wkv_d = nc.dram_tensor("wkv", [C, 2 * C], bf16, kind="ExternalInput").ap()
    w1_d = nc.dram_tensor("w1", [C, OUT], f32, kind="ExternalInput").ap()
    w2_d = nc.dram_tensor("w2", [C, OUT], f32, kind="ExternalInput").ap()
    mask_d = nc.dram_tensor("mask", [C, C], f32, kind="ExternalInput").ap()
    trid_d = nc.dram_tensor("trid", [C, C], f32, kind="ExternalInput").ap()
    out_d = nc.dram_tensor("out", [SPC, OUT, H, W], bf16, kind="ExternalOutput").ap()
    out_v = out_d.rearrange("s o h w -> s o (h w)")

    def conv_ap(t, c8, a, b, dx):
        """DoubleRow ifmap: [C, 2@(b-a)*PW, 8@PW, 64@1] window of a [C,PH,PW] tile."""
        ap = t[:].copy()
        part = ap.ap[0]
        ap.ap = bass_rust.VecI64Pair(
            [list(part), [(b - a) * PW, 2], [PW, 8], [1, 64]]
        )
        ap.offset = t[:].offset + (8 * c8 + a) * PW + dx
        return ap

    with tile.TileContext(nc) as tc, __import__("contextlib").ExitStack() as ctx:
        wpool = ctx.enter_context(tc.tile_pool(name="w", bufs=1))
        srcp_pool = ctx.enter_context(tc.tile_pool(name="srcp", bufs=2))
        srcp8_pool = ctx.enter_context(tc.tile_pool(name="srcp8", bufs=2))
        s_pool = ctx.enter_context(tc.tile_pool(name="s", bufs=12))
        q_pool = ctx.enter_context(tc.tile_pool(name="q", bufs=16))
        r1_pool = ctx.enter_context(tc.tile_pool(name="r1", bufs=10))
        vpad_pool = ctx.enter_context(tc.tile_pool(name="vpad", bufs=2))
        eT_pool = ctx.enter_context(tc.tile_pool(name="eT", bufs=2))
        vT_pool = ctx.enter_context(tc.tile_pool(name="vT", bufs=2))
        tmp_pool = ctx.enter_context(tc.tile_pool(name="tmp", bufs=2))
        stage_pool = ctx.enter_context(tc.tile_pool(name="stage", bufs=4))
        small_pool = ctx.enter_context(tc.tile_pool(name="small", bufs=2))
        ps_pool = ctx.enter_context(tc.tile_pool(name="ps", bufs=4, space="PSUM"))
        grp_pool = ctx.enter_context(tc.tile_pool(name="grp", bufs=4, space="PSUM"))

        # ---- DMA: weights + src in order of first use; src in quarters ----
        d3_sb = wpool.tile([C, len(PAIRS3), 2, C], f8)
        nc.sync.dma_start(d3_sb[:], d3_d[:])
        warm = wpool.tile([C, 2], f32)
        nc.vector.memset(warm[:], 0.0)
        nc.scalar.activation(warm[:, 0:1], warm[:, 1:2], Act.Copy)
        nc.scalar.activation(warm[:, 0:1], warm[:, 1:2], Act.Exp)
        nc.scalar.activation(warm[:, 0:1], warm[:, 1:2], Act.Sigmoid)
        srcp_t, srcp8_t, pool_sums = [], [], []
        for smp in range(SPC):
            srcp = srcp_pool.tile([C, H, W], f32, tag="srcp")
            eng = nc.sync
            for qi in range(4):
                eng.dma_start(
                    r(srcp[:, 16 * qi : 16 * qi + 16, :]),
                    r(src_d[smp, :, 16 * qi : 16 * qi + 16]),
                )
            srcp_t.append(srcp)
            srcp8 = srcp8_pool.tile([C, PH, PW], f8, tag="srcp8")
            nc.gpsimd.memset(srcp8[:, 0:1, :], 0.0)
            nc.gpsimd.memset(srcp8[:, H + 1 : PH, :], 0.0)
            nc.gpsimd.memset(srcp8[:, 1 : H + 1, 0:1], 0.0)
            nc.gpsimd.memset(srcp8[:, 1 : H + 1, W + 1 : PW], 0.0)
            for qi in range(8):
                eng = nc.vector if qi % 2 == 0 else nc.scalar
                if eng is nc.scalar:
                    eng.activation(
                        srcp8[:, 1 + 8 * qi : 1 + 8 * qi + 8, 1 : W + 1],
                        srcp[:, 8 * qi : 8 * qi + 8, :], Act.Copy,
                    )
                else:
                    eng.tensor_copy(
                        srcp8[:, 1 + 8 * qi : 1 + 8 * qi + 8, 1 : W + 1],
                        srcp[:, 8 * qi : 8 * qi + 8, :],
                    )
            srcp8_t.append(srcp8)
            if smp == 0:
                wq_sb = wpool.tile([C, C], bf16)
                nc.sync.dma_start(wq_sb[:], wq_d[:])
                wv_sb = wpool.tile([C, C], bf16)
                nc.sync.dma_start(wv_sb[:], wv_d[:])
                wkv_sb = wpool.tile([C, 2 * C], bf16)
                nc.sync.dma_start(wkv_sb[:], wkv_d[:])
        d5_sb = wpool.tile([C, len(PAIRS5), 2, C], f8)
        nc.sync.dma_start(d5_sb[:], d5_d[:])
        mask_sb = wpool.tile([C, C], f32)
        nc.sync.dma_start(mask_sb[:], mask_d[:])
        trid_sb = wpool.tile([C, C], f32)
        nc.sync.dma_start(trid_sb[:], trid_d[:])
        w1_sb = wpool.tile([C, OUT], f32)
        nc.sync.dma_start(r(w1_sb[:]), r(w1_d[:]))
        w2_sb = wpool.tile([C, OUT], f32)
        nc.sync.dma_start(w2_sb[:], w2_d[:])

        # ---- phase generators; two samples zipped so evac-gated stretches of
        # one sample are filled by compute-bound matmuls of the other ----
        state = [dict() for _ in range(SPC)]

        def gen_cpe(smp):
            srcp, srcp8 = srcp_t[smp], srcp8_t[smp]
            s_t = state[smp]["s_t"] = []
            for g, chunks in enumerate(((0, 1), (2, 3, 4), (5, 6, 7))):
                ps3 = [grp_pool.tile([C, 512], f32, tag="grp", name=f"ps3_{smp}_{g}_{i}")
                       for i in range(len(chunks))]
                for p, (a, b, dx) in enumerate(PAIRS3):
                    for ci, c8 in enumerate(chunks):
                        nc.tensor.matmul(
                            ps3[ci][:], d3_sb[:, p, :, :],
                            conv_ap(srcp8, c8, a, b, dx),
                            start=(p == 0), stop=(p == len(PAIRS3) - 1),
                            perf_mode=DR,
                        )
                    yield
                for ci, c8 in enumerate(chunks):
                    st = s_pool.tile([C, 512], bf16, tag="s", name=f"st_{smp}_{c8}")
                    nc.vector.scalar_tensor_tensor(
                        st[:], ps3[ci][:], 1.0 / W3S,
                        srcp[:, 8 * c8 : 8 * c8 + 8, :],
                        Alu.mult, Alu.add,
                    )
                    s_t.append(st)
                yield

        def gen_qv(smp):
            srcp = srcp_t[smp]
            s_t = state[smp]["s_t"]
            vpad = state[smp]["vpad"] = vpad_pool.tile([C, PH, PW], f8, tag="vpad",
                                                       name=f"vpad{smp}")
            nc.vector.memset(vpad[:, 0:2, :], 0.0)
            nc.vector.memset(vpad[:, H + 2 : PH, :], 0.0)
            nc.vector.memset(vpad[:, 2 : H + 2, 0:2], 0.0)
            nc.vector.memset(vpad[:, 2 : H + 2, W + 2 : PW], 0.0)
            q_t = state[smp]["q_t"] = []
            yield
            for c8 in range(NC8):
                psq = ps_pool.tile([C, 512], f32, tag="ps")
                nc.tensor.matmul(psq[:], wq_sb[:], s_t[c8][:], start=True, stop=True)
                qt = q_pool.tile([C, 512], f32, tag="q", name=f"qt_{smp}_{c8}")
                nc.scalar.activation(r(qt[:]), psq[:], Act.Copy, scale=1.0 / QS)
                q_t.append(qt)
                psv = ps_pool.tile([C, 512], f32, tag="ps")
                nc.tensor.matmul(psv[:], wv_sb[:], s_t[c8][:], start=True, stop=True)
                if c8 % 2 == 0:
                    nc.scalar.activation(
                        vpad[:, 2 + 8 * c8 : 2 + 8 * c8 + 8, 2 : W + 2],
                        psv[:].rearrange("p (a b) -> p a b", a=8), Act.Copy,
                    )
                else:
                    nc.vector.tensor_copy(
                        vpad[:, 2 + 8 * c8 : 2 + 8 * c8 + 8, 2 : W + 2],
                        psv[:].rearrange("p (a b) -> p a b", a=8),
                    )
                yield
            pool_sum = small_pool.tile([C, 1], f32, tag="psum_vec", name=f"pool_sum{smp}")
            nc.vector.reduce_sum(pool_sum[:], srcp[:], axis=mybir.AxisListType.XY)
            state[smp]["pool_sum"] = pool_sum
            yield

        def gen_kvT_cl(smp):
            s_t = state[smp]["s_t"]
            eT = eT_pool.tile([C, NJ, C], bf16, tag="eT", name=f"eT{smp}")
            vT = vT_pool.tile([C, NJ, C + 1], bf16, tag="vT", name=f"vT{smp}")
            nc.vector.memset(vT[:, :, C : C + 1], 1.0)
            for jj in range(NJ // 2):
                psT = ps_pool.tile([C, 2, 2 * C], f32, tag="ps")
                for h in range(2):
                    j = 2 * jj + h
                    lhs = s_t[j // 4][:, (j % 4) * 128 : (j % 4 + 1) * 128]
                    nc.tensor.matmul(psT[:, h, :], lhs, wkv_sb[:], start=True, stop=True)
                nc.scalar.activation(eT[:, 2 * jj : 2 * jj + 2, :], psT[:, :, 0:C], Act.Exp)
                nc.vector.tensor_copy(vT[:, 2 * jj : 2 * jj + 2, 0:C], psT[:, :, C : 2 * C])
                yield
            ps_eca = ps_pool.tile([C, 1], f32, tag="ps")
            nc.tensor.matmul(ps_eca[:], trid_sb[:], state[smp]["pool_sum"][:],
                             start=True, stop=True)
            ca = small_pool.tile([C, 1], f32, tag="ca", name=f"ca{smp}")
            nc.scalar.activation(ca[:], ps_eca[:], Act.Sigmoid)
            w2p = state[smp]["w2p"] = stage_pool.tile([C, OUT], f32, tag="w2p",
                                                      name=f"w2p{smp}")
            nc.vector.tensor_scalar(r(w2p[:]), w2_sb[:], ca[:], None, Alu.mult)
            yield
            ps_cl = ps_pool.tile([C, C + 1], f32, tag="ps")
            for j in range(NJ):
                nc.tensor.matmul(
                    ps_cl[:], eT[:, j, :], vT[:, j, :],
                    start=(j == 0), stop=(j == NJ - 1),
                )
                if j % 8 == 7:
                    yield
            recip = small_pool.tile([C, 1], f32, tag="recip", name=f"recip{smp}")
            nc.vector.reciprocal(recip[:], ps_cl[:, C : C + 1])
            cln_t = small_pool.tile([C, C], f32, tag="cln_t", name=f"cln_t{smp}")
            nc.vector.tensor_scalar(cln_t[:], ps_cl[:, 0:C], recip[:], None, Alu.mult)
            cln = state[smp]["cln"] = small_pool.tile([C, C], f32, tag="cln",
                                                      name=f"cln{smp}")
            nc.vector.tensor_tensor(r(cln[:]), cln_t[:], mask_sb[:], Alu.mult)
            yield

        def gen_5x5(smp):
            vpad = state[smp]["vpad"]
            q_t = state[smp]["q_t"]
            cln = state[smp]["cln"]
            r1_t = state[smp]["r1_t"] = [None] * NC8
            for g in range(2):
                ps5 = [grp_pool.tile([C, 512], f32, tag="grp", name=f"ps5_{smp}_{g}_{i}")
                       for i in range(4)]
                for p, (a, b, dx) in enumerate(PAIRS5):
                    for ci in range(4):
                        nc.tensor.matmul(
                            ps5[ci][:], d5_sb[:, p, :, :],
                            conv_ap(vpad, g * 4 + ci, a, b, dx),
                            start=(p == 0), stop=(p == len(PAIRS5) - 1),
                            perf_mode=DR,
                        )
                    yield
                for ci in range(4):
                    c8 = g * 4 + ci
                    psc = ps_pool.tile([C, 512], f32, tag="ps")
                    nc.tensor.matmul(psc[:], r(cln[:]), r(q_t[c8][:]), start=True, stop=True)
                    tmp = tmp_pool.tile([C, 512], f32, tag="tmp")
                    nc.vector.tensor_tensor(tmp[:], q_t[c8][:], ps5[ci][:], Alu.mult)
                    rt = r1_pool.tile([C, 512], f32, tag="r1", name=f"rt_{smp}_{c8}")
                    nc.vector.tensor_tensor(r(rt[:]), tmp[:], psc[:], Alu.add)
                    r1_t[c8] = rt
                    yield

        def gen_outproj(smp):
            srcp = srcp_t[smp]
            r1_t = state[smp]["r1_t"]
            w2p = state[smp]["w2p"]
            dma_engs = [nc.sync, nc.scalar, nc.gpsimd]
            evac_engs = [nc.scalar, nc.vector]
            for half in range(2):
                for m in range(OUT // C):
                    stg = stage_pool.tile([C, 4, 512], bf16, tag="stage")
                    for cc in range(4):
                        c8 = half * 4 + cc
                        pso = ps_pool.tile([C, 512], f32, tag="ps")
                        nc.tensor.matmul(
                            pso[:], r(w1_sb[:, m * C : (m + 1) * C]), r(r1_t[c8][:]),
                            start=True, stop=False,
                        )
                        y0 = 8 * c8
                        nc.tensor.matmul(
                            pso[:], r(w2p[:, m * C : (m + 1) * C]),
                            r(srcp[:, y0 : y0 + 8, :]),
                            start=False, stop=True,
                        )
                        eng = evac_engs[(m * 8 + half * 4 + cc) % 2]
                        if eng is nc.scalar:
                            eng.activation(stg[:, cc, :], pso[:], Act.Copy)
                        else:
                            eng.tensor_copy(stg[:, cc, :], pso[:])
                    if smp == SPC - 1 and half == 1 and m >= 2:
                        for dd in range(2):
                            dma_engs[dd].dma_start(
                                out_v[smp, m * C : (m + 1) * C,
                                      half * 2048 + dd * 1024 : half * 2048 + dd * 1024 + 1024],
                                stg[:, 2 * dd : 2 * dd + 2, :].rearrange("p a b -> p (a b)"),
                            )
                    else:
                        engs = dma_engs if half == 0 else dma_engs[:2]
                        engs[(m * 2 + half) % len(engs)].dma_start(
                            out_v[smp, m * C : (m + 1) * C, half * 2048 : (half + 1) * 2048],
                            stg[:].rearrange("p a b -> p (a b)"),
                        )
                    yield

        def run_all(g):
            for _ in g:
                pass

        def zip_emit(ga, gb, na, nb):
            while True:
                done_a = done_b = False
                for _ in range(na):
                    try:
                        next(ga)
                    except StopIteration:
                        done_a = True
                        break
                for _ in range(nb):
                    try:
                        next(gb)
                    except StopIteration:
                        done_b = True
                        break
                if done_a:
                    run_all(gb)
                    return
                if done_b:
                    run_all(ga)
                    return

        run_all(gen_cpe(0))
        zip_emit(gen_qv(0), gen_cpe(1), 1, 2)
        zip_emit(gen_kvT_cl(0), gen_qv(1), 2, 1)
        zip_emit(gen_5x5(0), gen_kvT_cl(1), 2, 1)
        zip_emit(gen_outproj(0), gen_5x5(1), 1, 5)
        run_all(gen_outproj(1))
    nc.compile()
    return nc


def _get_nc():
    if "nc" not in _CACHE:
        _CACHE["nc"] = _build_nc()
    return _CACHE["nc"]


def _host_weights(cpe_w, qkv_w, rel_pos, conv1d_w, out_w):
    cpe_w = np.asarray(cpe_w, np.float32)
    qkv_w = np.asarray(qkv_w, np.float32)
    rel_pos = np.asarray(rel_pos, np.float32)
    conv1d_w = np.asarray(conv1d_w, np.float32)
    out_w = np.asarray(out_w, np.float32)
    idx = np.arange(C)
    f8 = ml_dtypes.float8_e4m3

    def pack_pairs(w, pairs, scale):
        # w: [C, KH, KW] per-channel taps -> [C, npairs, 2, C] fp8 diag pairs
        out = np.zeros([C, len(pairs), 2, C], np.float32)
        for p, (a, b, dx) in enumerate(pairs):
            if a == b:
                out[idx, p, 0, idx] = w[:, a, dx] * (scale * 0.5)
                out[idx, p, 1, idx] = w[:, a, dx] * (scale * 0.5)
            else:
                out[idx, p, 0, idx] = w[:, a, dx] * scale
                out[idx, p, 1, idx] = w[:, b, dx] * scale
        return out.astype(f8)

    d3 = pack_pairs(cpe_w[:, 0, :, :], PAIRS3, W3S)
    d5 = pack_pairs(np.tile(rel_pos, (NH, 1, 1)), PAIRS5, QS)

    wq = np.ascontiguousarray(qkv_w[0:C, :].T).astype(ml_dtypes.bfloat16)
    wv = np.ascontiguousarray(qkv_w[2 * C : 3 * C, :].T).astype(ml_dtypes.bfloat16)
    wkv = np.ascontiguousarray(qkv_w[C : 3 * C, :].T).astype(ml_dtypes.bfloat16)
    w1 = np.ascontiguousarray(out_w[:, 0:C].T)
    w2 = np.ascontiguousarray(out_w[:, C : 2 * C].T)

    mask = np.zeros([C, C], np.float32)
    for h in range(NH):
        mask[h * HD : (h + 1) * HD, h * HD : (h + 1) * HD] = SCALING * QS

    trid = np.zeros([C, C], np.float32)
    trid[idx[:-1], idx[:-1] + 1] = conv1d_w[0]  # pool[c-1] contributes to ca[c]
    trid[idx, idx] = conv1d_w[1]
    trid[idx[1:], idx[1:] - 1] = conv1d_w[2]
    trid *= 1.0 / N
    return dict(d3=d3, d5=d5, wq=wq, wv=wv, wkv=wkv, w1=w1, w2=w2,
                mask=mask, trid=trid)


def kernel(src, cpe_w, qkv_w, rel_pos, conv1d_w, out_w):
    from concourse.bass_utils import run_bass_kernel_spmd

    src = np.asarray(src, np.float32)
    w = _host_weights(cpe_w, qkv_w, rel_pos, conv1d_w, out_w)
    nc = _get_nc()
    in_maps = [
        {"src": np.ascontiguousarray(src[i * SPC : (i + 1) * SPC]), **w}
        for i in range(NCORES)
    ]
    trace = bool(os.environ.get("BASS_TRACE"))
    res = run_bass_kernel_spmd(nc, in_maps, list(range(NCORES)), trace=trace)
    _CACHE["last_result"] = res
    out = np.concatenate(
        [np.asarray(res.results[i]["out"], np.float32) for i in range(NCORES)], axis=0
    )
    return out


# revision 24
# speedup vs baseline: 1.0044x; 1.0044x over previous
# Bass/Trainium2 kernel for MHConvAttention (B=16, C=128, H=W=64, NH=8, OUT=512)
# Data-parallel over batch: 8 cores x 2 samples each.
#
# Per-sample layout: channels (128) on SBUF partitions, flattened spatial (4096)
# on the free dim. Depthwise convs run as fp8 DoubleRow matmuls: two taps packed
# per PE cell (pair stride in the padded-row buffer is 80 B, a multiple of 16),
# tap-major over 4-chunk groups so LDWEIGHTS hides behind the 4 streams. The
# content-lambda path uses a transposed QKV GEMM; the ECA channel-attention is
# folded into the out-projection weights; output is DMA'd in bf16.
import os
import numpy as np
import ml_dtypes

B, C, H, W = 16, 128, 64, 64
NH, HD, WIN, OUT = 8, 16, 5, 512
N = H * W
NCORES = 8
SPC = B // NCORES          # samples per core
NC8 = N // 512             # 512-wide chunks per sample
NJ = N // 128              # 128-wide chunks (transposed GEMM)
SCALING = HD ** (-0.5)
PW = 80                    # padded row width (fp8 conv buffers); 80 % 16 == 0
PH = 72                    # padded rows
QS = 16.0                  # q is stored as q/QS; conv5 weights carry x QS
W3S = 8.0                  # cpe weights stored x W3S in fp8; dequant 1/W3S
# tap pairs (dy_a, dy_b, dx); dy_a == dy_b means a half-weight self-pair
PAIRS3 = [(a, b, dx) for dx in range(3) for (a, b) in ((0, 1), (2, 2))]
PAIRS5 = [(a, b, dx) for dx in range(5) for (a, b) in ((0, 1), (2, 3), (4, 4))]

_CACHE = {}


def _build_nc():
    import bass_rust
    import concourse.bass as bass
    import concourse.tile as tile
    import concourse.mybir as mybir
    from concourse import bacc

    f32 = mybir.dt.float32
    f32r = mybir.dt.float32r
    bf16 = mybir.dt.bfloat16
    f8 = mybir.dt.float8e4
    Alu = mybir.AluOpType
    Act = mybir.ActivationFunctionType
    DR = mybir.MatmulPerfMode.DoubleRow

    def r(ap):
        return ap.bitcast(f32r)

    nc = bacc.Bacc(trn_type="TRN2", target_bir_lowering=False, debug=False)

    src_d = nc.dram_tensor("src", [SPC, C, H, W], f32, kind="ExternalInput").ap()
    d3_d = nc.dram_tensor("d3", [C, len(PAIRS3), 2, C], f8, kind="ExternalInput").ap()
    d5_d = nc.dram_tensor("d5", [C, len(PAIRS5), 2, C], f8, kind="ExternalInput").ap()
    wq_d = nc.dram_tensor("wq", [C, C], bf16, kind="ExternalInput").ap()
    wv_d = nc.dram_tensor("wv", [C, C], bf16, kind="ExternalInput").ap()
    wkv_d = nc.dram_tensor("wkv", [C, 2 * C], bf16, kind="ExternalInput").ap()
    w1_d = nc.dram_tensor("w1", [C, OUT], f32, kind="ExternalInput").ap()
    w2_d = nc.dram_tensor("w2", [C, OUT], f32, kind="ExternalInput").ap()
    mask_d = nc.dram_tensor("mask", [C, C], f32, kind="ExternalInput").ap()
    trid_d = nc.dram_tensor("trid", [C, C], f32, kind="ExternalInput").ap()
    out_d = nc.dram_tensor("out", [SPC, OUT, H, W], bf16, kind="ExternalOutput").ap()
    out_v = out_d.rearrange("s o h w -> s o (h w)")

    def conv_ap(t, c8, a, b, dx):
        """DoubleRow ifmap: [C, 2@(b-a)*PW, 8@PW, 64@1] window of a [C,PH,PW] tile."""
        ap = t[:].copy()
        part = ap.ap[0]
        ap.ap = bass_rust.VecI64Pair(
            [list(part), [(b - a) * PW, 2], [PW, 8], [1, 64]]
        )
        ap.offset = t[:].offset + (8 * c8 + a) * PW + dx
        return ap

    with tile.TileContext(nc) as tc, __import__("contextlib").ExitStack() as ctx:
        wpool = ctx.enter_context(tc.tile_pool(name="w", bufs=1))
        srcp_pool = ctx.enter_context(tc.tile_pool(name="srcp", bufs=2))
        srcp8_pool = ctx.enter_context(tc.tile_pool(name="srcp8", bufs=2))
        s_pool = ctx.enter_context(tc.tile_pool(name="s", bufs=12))
        q_pool = ctx.enter_context(tc.tile_pool(name="q", bufs=16))
        r1_pool = ctx.enter_context(tc.tile_pool(name="r1", bufs=10))
        vpad_pool = ctx.enter_context(tc.tile_pool(name="vpad", bufs=2))
        eT_pool = ctx.enter_context(tc.tile_pool(name="eT", bufs=2))
        vT_pool = ctx.enter_context(tc.tile_pool(name="vT", bufs=2))
        tmp_pool = ctx.enter_context(tc.tile_pool(name="tmp", bufs=2))
        stage_pool = ctx.enter_context(tc.tile_pool(name="stage", bufs=4))
        small_pool = ctx.enter_context(tc.tile_pool(name="small", bufs=2))
        ps_pool = ctx.enter_context(tc.tile_pool(name="ps", bufs=4, space="PSUM"))
        grp_pool = ctx.enter_context(tc.tile_pool(name="grp", bufs=4, space="PSUM"))

        # ---- DMA: weights + src in order of first use; src in quarters ----
        d3_sb = wpool.tile([C, len(PAIRS3), 2, C], f8)
        nc.scalar.dma_start(d3_sb[:], d3_d[:])
        warm = wpool.tile([C, 2], f32)
        nc.vector.memset(warm[:], 0.0)
        nc.scalar.activation(warm[:, 0:1], warm[:, 1:2], Act.Copy)
        nc.scalar.activation(warm[:, 0:1], warm[:, 1:2], Act.Exp)
        nc.scalar.activation(warm[:, 0:1], warm[:, 1:2], Act.Sigmoid)
        srcp_t, srcp8_t, pool_sums = [], [], []
        for smp in range(SPC):
            srcp = srcp_pool.tile([C, H, W], f32, tag="srcp")
            eng = nc.sync
            for qi in range(4):
                eng.dma_start(
                    r(srcp[:, 16 * qi : 16 * qi + 16, :]),
                    r(src_d[smp, :, 16 * qi : 16 * qi + 16]),
                )
            srcp_t.append(srcp)
            srcp8 = srcp8_pool.tile([C, PH, PW], f8, tag="srcp8")
            nc.gpsimd.memset(srcp8[:, 0:1, :], 0.0)
            nc.gpsimd.memset(srcp8[:, H + 1 : PH, :], 0.0)
            nc.gpsimd.memset(srcp8[:, 1 : H + 1, 0:1], 0.0)
            nc.gpsimd.memset(srcp8[:, 1 : H + 1, W + 1 : PW], 0.0)
            for qi in range(8):
                eng = nc.vector if qi % 2 == 0 else nc.scalar
                if eng is nc.scalar:
                    eng.activation(
                        srcp8[:, 1 + 8 * qi : 1 + 8 * qi + 8, 1 : W + 1],
                        srcp[:, 8 * qi : 8 * qi + 8, :], Act.Copy,
                    )
                else:
                    eng.tensor_copy(
                        srcp8[:, 1 + 8 * qi : 1 + 8 * qi + 8, 1 : W + 1],
                        srcp[:, 8 * qi : 8 * qi + 8, :],
                    )
            srcp8_t.append(srcp8)
            if smp == 0:
                wq_sb = wpool.tile([C, C], bf16)
                nc.sync.dma_start(wq_sb[:], wq_d[:])
                wv_sb = wpool.tile([C, C], bf16)
                nc.sync.dma_start(wv_sb[:], wv_d[:])
                wkv_sb = wpool.tile([C, 2 * C], bf16)
                nc.sync.dma_start(wkv_sb[:], wkv_d[:])
        d5_sb = wpool.tile([C, len(PAIRS5), 2, C], f8)
        nc.sync.dma_start(d5_sb[:], d5_d[:])
        mask_sb = wpool.tile([C, C], f32)
        nc.sync.dma_start(mask_sb[:], mask_d[:])
        trid_sb = wpool.tile([C, C], f32)
        nc.sync.dma_start(trid_sb[:], trid_d[:])
        w1_sb = wpool.tile([C, OUT], f32)
        nc.sync.dma_start(r(w1_sb[:]), r(w1_d[:]))
        w2_sb = wpool.tile([C, OUT], f32)
        nc.sync.dma_start(w2_sb[:], w2_d[:])

        # ---- phase generators; two samples zipped so evac-gated stretches of
        # one sample are filled by compute-bound matmuls of the other ----
        state = [dict() for _ in range(SPC)]

        def gen_cpe(smp):
            srcp, srcp8 = srcp_t[smp], srcp8_t[smp]
            s_t = state[smp]["s_t"] = []
            for g, chunks in enumerate(((0, 1), (2, 3, 4), (5, 6, 7))):
                ps3 = [grp_pool.tile([C, 512], f32, tag="grp", name=f"ps3_{smp}_{g}_{i}")
                       for i in range(len(chunks))]
                for p, (a, b, dx) in enumerate(PAIRS3):
                    for ci, c8 in enumerate(chunks):
                        nc.tensor.matmul(
                            ps3[ci][:], d3_sb[:, p, :, :],
                            conv_ap(srcp8, c8, a, b, dx),
                            start=(p == 0), stop=(p == len(PAIRS3) - 1),
                            perf_mode=DR,
                        )
                    yield
                for ci, c8 in enumerate(chunks):
                    st = s_pool.tile([C, 512], bf16, tag="s", name=f"st_{smp}_{c8}")
                    nc.vector.scalar_tensor_tensor(
                        st[:], ps3[ci][:], 1.0 / W3S,
                        srcp[:, 8 * c8 : 8 * c8 + 8, :],
                        Alu.mult, Alu.add,
                    )
                    s_t.append(st)
                yield

        def gen_qv(smp):
            srcp = srcp_t[smp]
            s_t = state[smp]["s_t"]
            vpad = state[smp]["vpad"] = vpad_pool.tile([C, PH, PW], f8, tag="vpad",
                                                       name=f"vpad{smp}")
            nc.vector.memset(vpad[:, 0:2, :], 0.0)
            nc.vector.memset(vpad[:, H + 2 : PH, :], 0.0)
            nc.vector.memset(vpad[:, 2 : H + 2, 0:2], 0.0)
            nc.vector.memset(vpad[:, 2 : H + 2, W + 2 : PW], 0.0)
            q_t = state[smp]["q_t"] = []
            yield
            for c8 in range(NC8):
                psq = ps_pool.tile([C, 512], f32, tag="ps")
                nc.tensor.matmul(psq[:], wq_sb[:], s_t[c8][:], start=True, stop=True)
                qt = q_pool.tile([C, 512], f32, tag="q", name=f"qt_{smp}_{c8}")
                nc.scalar.activation(r(qt[:]), psq[:], Act.Copy, scale=1.0 / QS)
                q_t.append(qt)
                psv = ps_pool.tile([C, 512], f32, tag="ps")
                nc.tensor.matmul(psv[:], wv_sb[:], s_t[c8][:], start=True, stop=True)
                if c8 % 2 == 0:
                    nc.scalar.activation(
                        vpad[:, 2 + 8 * c8 : 2 + 8 * c8 + 8, 2 : W + 2],
                        psv[:].rearrange("p (a b) -> p a b", a=8), Act.Copy,
                    )
                else:
                    nc.vector.tensor_copy(
                        vpad[:, 2 + 8 * c8 : 2 + 8 * c8 + 8, 2 : W + 2],
                        psv[:].rearrange("p (a b) -> p a b", a=8),
                    )
                yield
            pool_sum = small_pool.tile([C, 1], f32, tag="psum_vec", name=f"pool_sum{smp}")
            nc.vector.reduce_sum(pool_sum[:], srcp[:], axis=mybir.AxisListType.XY)
            state[smp]["pool_sum"] = pool_sum
            yield

        def gen_kvT_cl(smp):
            s_t = state[smp]["s_t"]
            eT = eT_pool.tile([C, NJ, C], bf16, tag="eT", name=f"eT{smp}")
            vT = vT_pool.tile([C, NJ, C + 1], bf16, tag="vT", name=f"vT{smp}")
            nc.vector.memset(vT[:, :, C : C + 1], 1.0)
            for jj in range(NJ // 2):
                psT = ps_pool.tile([C, 2, 2 * C], f32, tag="ps")
                for h in range(2):
                    j = 2 * jj + h
                    lhs = s_t[j // 4][:, (j % 4) * 128 : (j % 4 + 1) * 128]
                    nc.tensor.matmul(psT[:, h, :], lhs, wkv_sb[:], start=True, stop=True)
                nc.scalar.activation(eT[:, 2 * jj : 2 * jj + 2, :], psT[:, :, 0:C], Act.Exp)
                nc.vector.tensor_copy(vT[:, 2 * jj : 2 * jj + 2, 0:C], psT[:, :, C : 2 * C])
                yield
            ps_eca = ps_pool.tile([C, 1], f32, tag="ps")
            nc.tensor.matmul(ps_eca[:], trid_sb[:], state[smp]["pool_sum"][:],
                             start=True, stop=True)
            ca = small_pool.tile([C, 1], f32, tag="ca", name=f"ca{smp}")
            nc.scalar.activation(ca[:], ps_eca[:], Act.Sigmoid)
            w2p = state[smp]["w2p"] = stage_pool.tile([C, OUT], f32, tag="w2p",
                                                      name=f"w2p{smp}")
            nc.vector.tensor_scalar(r(w2p[:]), w2_sb[:], ca[:], None, Alu.mult)
            yield
            ps_cl = ps_pool.tile([C, C + 1], f32, tag="ps")
            for j in range(NJ):
                nc.tensor.matmul(
                    ps_cl[:], eT[:, j, :], vT[:, j, :],
                    start=(j == 0), stop=(j == NJ - 1),
                )
                if j % 8 == 7:
                    yield
            recip = small_pool.tile([C, 1], f32, tag="recip", name=f"recip{smp}")
            nc.vector.reciprocal(recip[:], ps_cl[:, C : C + 1])
            cln_t = small_pool.tile([C, C], f32, tag="cln_t", name=f"cln_t{smp}")
            nc.vector.tensor_scalar(cln_t[:], ps_cl[:, 0:C], recip[:], None, Alu.mult)
            cln = state[smp]["cln"] = small_pool.tile([C, C], f32, tag="cln",
                                                      name=f"cln{smp}")
            nc.vector.tensor_tensor(r(cln[:]), cln_t[:], mask_sb[:], Alu.mult)
            yield

        def gen_5x5(smp):
            vpad = state[smp]["vpad"]
            q_t = state[smp]["q_t"]
            cln = state[smp]["cln"]
            r1_t = state[smp]["r1_t"] = [None] * NC8
            for g in range(2):
                ps5 = [grp_pool.tile([C, 512], f32, tag="grp", name=f"ps5_{smp}_{g}_{i}")
                       for i in range(4)]
                for p, (a, b, dx) in enumerate(PAIRS5):
                    for ci in range(4):
                        nc.tensor.matmul(
                            ps5[ci][:], d5_sb[:, p, :, :],
                            conv_ap(vpad, g * 4 + ci, a, b, dx),
                            start=(p == 0), stop=(p == len(PAIRS5) - 1),
                            perf_mode=DR,
                        )
                    yield
                for ci in range(4):
                    c8 = g * 4 + ci
                    psc = ps_pool.tile([C, 512], f32, tag="ps")
                    nc.tensor.matmul(psc[:], r(cln[:]), r(q_t[c8][:]), start=True, stop=True)
                    tmp = tmp_pool.tile([C, 512], f32, tag="tmp")
                    nc.vector.tensor_tensor(tmp[:], q_t[c8][:], ps5[ci][:], Alu.mult)
                    rt = r1_pool.tile([C, 512], f32, tag="r1", name=f"rt_{smp}_{c8}")
                    nc.vector.tensor_tensor(r(rt[:]), tmp[:], psc[:], Alu.add)
                    r1_t[c8] = rt
                    yield

        def gen_outproj(smp, split_evac=False):
            srcp = srcp_t[smp]
            r1_t = state[smp]["r1_t"]
            w2p = state[smp]["w2p"]
            dma_engs = [nc.sync, nc.scalar, nc.gpsimd]
            evac_engs = [nc.scalar, nc.vector]
            for half in range(2):
                for m in range(OUT // C):
                    stg = stage_pool.tile([C, 4, 512], bf16, tag="stage")
                    for cc in range(4):
                        c8 = half * 4 + cc
                        pso = ps_pool.tile([C, 512], f32, tag="ps")
                        nc.tensor.matmul(
                            pso[:], r(w1_sb[:, m * C : (m + 1) * C]), r(r1_t[c8][:]),
                            start=True, stop=False,
                        )
                        y0 = 8 * c8
                        nc.tensor.matmul(
                            pso[:], r(w2p[:, m * C : (m + 1) * C]),
                            r(srcp[:, y0 : y0 + 8, :]),
                            start=False, stop=True,
                        )
                        if split_evac:
                            nc.scalar.activation(stg[:, cc, 0:256], pso[:, 0:256], Act.Copy)
                            nc.vector.tensor_copy(stg[:, cc, 256:512], pso[:, 256:512])
                        else:
                            eng = evac_engs[(m * 8 + half * 4 + cc) % 2]
                            if eng is nc.scalar:
                                eng.activation(stg[:, cc, :], pso[:], Act.Copy)
                            else:
                                eng.tensor_copy(stg[:, cc, :], pso[:])
                    if smp == SPC - 1 and half == 1 and m >= 2:
                        for dd in range(2):
                            dma_engs[dd].dma_start(
                                out_v[smp, m * C : (m + 1) * C,
                                      half * 2048 + dd * 1024 : half * 2048 + dd * 1024 + 1024],
                                stg[:, 2 * dd : 2 * dd + 2, :].rearrange("p a b -> p (a b)"),
                            )
                    else:
                        engs = dma_engs if half == 0 else dma_engs[:2]
                        engs[(m * 2 + half) % len(engs)].dma_start(
                            out_v[smp, m * C : (m + 1) * C, half * 2048 : (half + 1) * 2048],
                            stg[:].rearrange("p a b -> p (a b)"),
                        )
                    yield

        def run_all(g):
            for _ in g:
                pass

        def zip_emit(ga, gb, na, nb):
            while True:
                done_a = done_b = False
                for _ in range(na):
                    try:
                        next(ga)
                    except StopIteration:
                        done_a = True
                        break
                for _ in range(nb):
                    try:
                        next(gb)
                    except StopIteration:
                        done_b = True
                        break
                if done_a:
                    run_all(gb)
                    return
                if done_b:
                    run_all(ga)
                    return

        run_all(gen_cpe(0))
        zip_emit(gen_qv(0), gen_cpe(1), 1, 2)
        zip_emit(gen_kvT_cl(0), gen_qv(1), 2, 1)
        zip_emit(gen_5x5(0), gen_kvT_cl(1), 2, 1)
        zip_emit(gen_outproj(0), gen_5x5(1), 1, 5)
        run_all(gen_outproj(1, split_evac=True))
    nc.compile()
    return nc


def _get_nc():
    if "nc" not in _CACHE:
        _CACHE["nc"] = _build_nc()
    return _CACHE["nc"]


def _host_weights(cpe_w, qkv_w, rel_pos, conv1d_w, out_w):
    cpe_w = np.asarray(cpe_w, np.float32)
    qkv_w = np.asarray(qkv_w, np.float32)
    rel_pos = np.asarray(rel_pos, np.float32)
    conv1d_w = np.asarray(conv1d_w, np.float32)
    out_w = np.asarray(out_w, np.float32)
    idx = np.arange(C)
    f8 = ml_dtypes.float8_e4m3

    def pack_pairs(w, pairs, scale):
        # w: [C, KH, KW] per-channel taps -> [C, npairs, 2, C] fp8 diag pairs
        out = np.zeros([C, len(pairs), 2, C], np.float32)
        for p, (a, b, dx) in enumerate(pairs):
            if a == b:
                out[idx, p, 0, idx] = w[:, a, dx] * (scale * 0.5)
                out[idx, p, 1, idx] = w[:, a, dx] * (scale * 0.5)
            else:
                out[idx, p, 0, idx] = w[:, a, dx] * scale
                out[idx, p, 1, idx] = w[:, b, dx] * scale
        return out.astype(f8)

    d3 = pack_pairs(cpe_w[:, 0, :, :], PAIRS3, W3S)
    d5 = pack_pairs(np.tile(rel_pos, (NH, 1, 1)), PAIRS5, QS)

    wq = np.ascontiguousarray(qkv_w[0:C, :].T).astype(ml_dtypes.bfloat16)
    wv = np.ascontiguousarray(qkv_w[2 * C : 3 * C, :].T).astype(ml_dtypes.bfloat16)
    wkv = np.ascontiguousarray(qkv_w[C : 3 * C, :].T).astype(ml_dtypes.bfloat16)
    w1 = np.ascontiguousarray(out_w[:, 0:C].T)
    w2 = np.ascontiguousarray(out_w[:, C : 2 * C].T)

    mask = np.zeros([C, C], np.float32)
    for h in range(NH):
        mask[h * HD : (h + 1) * HD, h * HD : (h + 1) * HD] = SCALING * QS

    trid = np.zeros([C, C], np.float32)
    trid[idx[:-1], idx[:-1] + 1] = conv1d_w[0]  # pool[c-1] contributes to ca[c]
    trid[idx, idx] = conv1d_w[1]
    trid[idx[1:], idx[1:] - 1] = conv1d_w[2]
    trid *= 1.0 / N
    return dict(d3=d3, d5=d5, wq=wq, wv=wv, wkv=wkv, w1=w1, w2=w2,
                mask=mask, trid=trid)


def kernel(src, cpe_w, qkv_w, rel_pos, conv1d_w, out_w):
    from concourse.bass_utils import run_bass_kernel_spmd

    src = np.asarray(src, np.float32)
    w = _host_weights(cpe_w, qkv_w, rel_pos, conv1d_w, out_w)
    nc = _get_nc()
    in_maps = [
        {"src": np.ascontiguousarray(src[i * SPC : (i + 1) * SPC]), **w}
        for i in range(NCORES)
    ]
    trace = bool(os.environ.get("BASS_TRACE"))
    res = run_bass_kernel_spmd(nc, in_maps, list(range(NCORES)), trace=trace)
    _CACHE["last_result"] = res
    out = np.concatenate(
        [np.asarray(res.results[i]["out"], np.float32) for i in range(NCORES)], axis=0
    )
    return out
